# revision 55
# baseline (speedup 1.0000x reference)
"""Sharded sparse (windowed) attention for TRN2 — 8 NeuronCores, head-parallel.

Reference computation (B=4, N=197, C=2048, H=32 heads, hd=64, window=8):
    qkv = x @ qkv_w.T -> split q,k,v per head
    attn = softmax(mask_weight * (q@k.T) * hd^-0.5  with off-band -inf)
    out  = (attn @ v) per head, concat heads, @ proj_w.T + proj_b
Sharding: 4 heads per core (tensor parallel). Each core computes its heads'
qkv projection, windowed attention, and a partial of the output projection
(contraction over its 256 head-dims). Host sums the 8 partials + bias.

On-device layout is transposed (feature dim on partitions, tokens on the
free axis) until the output projection, which flips to (tokens, features):
    xT (2048, 788)  qkT (512, 788)  v (tokens, 256)  E=(j,i)  out (788, 2048)

Structure (measured ~76.9us HW exec, rel err 7.2e-4; earlier sessions:
v0 ~108us, v1 ~83.2us):
  - ALL input rides the sync HWDGE ring in consumption order: one queue's
    16 SDMA engines saturate the ~340-380 GB/s HBM rate alone, and the
    single FIFO delivers kc chunks at ~1.05-1.4us each (phase 1 consumes
    1.31us/kc). Singleton groups early (a group's semaphore fires when the
    SLOWEST engine finishes, so big groups stall the consumer), pairs late.
    masks+pw packed as ONE trailing transfer = arrive after all xw bytes.
  - 3 + 1 f32 warm-up matmuls bridge the PE from the ~7.5us framework
    preamble to kc0's arrival (~11us): an idle gap there resets the HAM
    ramp and phase 1 then runs at half clock for several microseconds;
    extra warm-ups would queue ahead of phase 1 on the in-order Tensor
    engine and delay it past its data.
  - phase 1 (q,k) ni-inner: 8 PSUM tiles across all 8 banks; ~164ns/matmul
    issue stride (fully pipelined, stream-rate-bound at 2.4GHz)
  - score blocks DISJOINT in i (no PSUM accumulation): blk0 i<120,
    blk1 i>=120 (its j rows [69,197) cover the window for those queries)
  - each head has its OWN score PSUM tile: the two heads of a pair contract
    on different PE row groups (partitions 0:64 / 64:128), and
    different-row-group matmuls draining into the SAME PSUM bank abort the
    device; adjacent score matmuls alternate row group AND bank, so they
    execute concurrently in the array
  - chain ops run once per head PAIR on [128, 396] tiles (mask-add merges
    the two score tiles into one wide SBUF tile; mask-mult at 2x DVE rate;
    one exp); packed zt [65, 396] is safe (AV matmuls are full-array)
  - only the softmax denominator row is staged to SBUF (the custom-DVE
    reciprocal reads garbage from PSUM on HW); z itself is normalized
    straight out of PSUM; mask pad columns are (M=0, A=0) so e=1 there and
    the packed reciprocal stays finite
  - output projection is YT-stationary: out[tok, 2048] = YT_chunk^T @ pw
    over 7 OVERLAPPING 128-row token chunks (starts PROJC0): always-128
    rows so every output DMA sprays all 16 SDMA engines (a 69-partition
    dma_start lands on ~3 engines and drains 4x slower); overlap rows are
    recomputed bitwise-identically so double-writing dram is safe
  - chunk ci is scheduled one slot AFTER the last chain covering its
    tokens (same-slot k2=1 matmuls block the in-order PE queue on the
    chain's YT write); chunks 3+4 run ahead of chain (3,0) so their
    evacuations clear Scalar/Vector before the FINAL chain needs them;
    chunk 5's k2=0 halves pend across the final chain (PSUM "mm"+"st");
    dummy matmuls on the final chain's sm tile fill the exp->AV wait and
    post-output dummies on obt[6] span the drain, holding the HAM clock
    at full rate through the tail and teardown
  - out partials fp16 (host accumulates in f32)
"""

import numpy as np

B = 4
N = 197
C = 2048
H = 32
HD = 64
WIN = 8
NCORES = 8
HPC = H // NCORES          # heads per core
CPC = HPC * HD             # head-dims per core (256)
T = B * N                  # 788 tokens
TP = T + 2                 # padded qkT width (block-1 rhs reads col 788)
KC = C // 128              # 16 contraction chunks
SCALE = HD ** -0.5
NEG = -200.0               # additive off-band mask (exp underflows to 0)

# banded blocks of ST[j, i]: (j0, jh, i0, iw, packed column offset)
# blocks are 128 j-rows and DISJOINT in i: block 0 handles queries i<120
# (window j<=127 fits its rows), block 1 handles i>=120 (window j>=112 fits
# its rows j in [69,197)). No overlap -> no PSUM accumulation between them.
BLOCKS = [(0, 128, 0, 120, 0), (69, 128, 120, 78, 120)]
SW = 198                   # packed score-tile width per head (120 + 78)
SW2 = 2 * SW               # two heads side by side
NP = 198                   # padded zt width
TOKCH = [(0, 128), (69, 128)]          # per-batch token chunks (v rows)
QKCH = [(0, 394), (394, 394)]          # qk token halves
# proj token chunks: 7 OVERLAPPING 128-row chunks covering all 788 tokens.
# Always 128 rows so every output DMA sprays all 16 SDMA engines (a
# partial-partition dma_start lands on only ~3 engines: ~4x slower drain).
# Overlap rows are recomputed from the same YT/pw inputs -> bitwise
# identical -> double-writing them to dram is safe.
PROJC0 = [0, 128, 256, 384, 463, 591, 660]
CW = 128
NS = 4                     # proj output column slices (2048 / 512)
# input groups: ALL on the sync HWDGE ring. One queue's 16 SDMA engines
# saturate the ~380 GB/s HBM rate alone, and a single FIFO delivers chunks
# in exactly consumption order at 1.05us/kc (phase 1 eats 1.31us/kc).
# Splitting across both rings halves each ring's rate to ~2.1us/kc and
# stalls the PE. A group's semaphore fires only when the SLOWEST of the 16
# engines finishes the whole group, so groups are singles early (arrival-
# critical) and pairs late (first 8 starts issue ungated; later ones wait
# on semaphore reuse, which is harmless since they transfer last anyway).
XGROUPS = [[0], [1], [2], [3], [4], [5], [6], [7],
           [8, 9], [10, 11], [12, 13], [14, 15]]
EXTW = 2 * SW2 + 2 * C     # masks + pw packed as one trailing transfer
NWARM = 3                  # f32 warm-up matmuls (bridge PE to first input:
                           # kc0 usable ~11.1us; a PE gap before that delays
                           # the HAM clock unthrottle and phase 1 runs at
                           # half clock for ~5us)

# Measured configs (HW exec / end-to-end relative error):
#   fp16/fp16: ~83.5us / 7.2e-4   bf16/bf16: ~86us / 5.8e-3
DT_BIG = "fp16"            # qkv + proj matmul operand dtype: fp16|f32r|bf16
DT_ATT = "fp16"            # attention matmul operand dtype:  fp16|f32r|bf16

_compiled = {}


def _dt(mybir, name):
    return {"f32r": mybir.dt.float32r, "bf16": mybir.dt.bfloat16,
            "fp16": mybir.dt.float16}[name]


def _build_program(dt_big, dt_att):
    import concourse.mybir as mybir
    import concourse.tile as tile
    from concourse import bacc

    F32 = mybir.dt.float32
    DTB = _dt(mybir, dt_big)
    DTA = _dt(mybir, dt_att)
    assert DTA == DTB, "masks+pw ride one packed transfer"

    nc = bacc.Bacc("TRN2", target_bir_lowering=False, debug=False)

    # xw = [xT | wqkT | wvT] packed on the 2048-row contraction axis
    XWW = T + 2 * CPC + CPC
    xw = nc.dram_tensor("xw", [C, XWW], DTB, kind="ExternalInput")
    # ext = [maskA | maskM | pw0 | pw1] in one transfer (fp16 == DTA == DTB)
    ext = nc.dram_tensor("ext", [128, EXTW], DTB, kind="ExternalInput")
    out_d = nc.dram_tensor("out", [T, C], DTB, kind="ExternalOutput")

    with tile.TileContext(nc) as tc:
        with (
            tc.tile_pool(name="persist", bufs=1) as per,
            tc.tile_pool(name="work", bufs=8) as wk,
            tc.tile_pool(name="ps", bufs=2, space="PSUM") as pp,
        ):
            # ---- input loads first: xw chunks in consumption order ----
            xwt = []            # kc -> [128, XWW] view
            for g in XGROUPS:
                gw = len(g) * XWW
                gt = per.tile([128, gw], DTB, tag=f"xwg{g[0]}")
                src = xw[g[0] * 128:(g[0] + len(g)) * 128, :]
                nc.sync.dma_start(
                    out=gt.rearrange("p (n c) -> p n c", n=len(g)),
                    in_=src.rearrange("(n p) c -> p n c", p=128))
                for i, kc in enumerate(g):
                    xwt.append(gt[:, i * XWW:(i + 1) * XWW])
            xt = [t[:, 0:T] for t in xwt]
            wqk_t = [t[:, T:T + 2 * CPC] for t in xwt]
            wv_t = [t[:, T + 2 * CPC:XWW] for t in xwt]

            # ---- masks + pw: one transfer, LAST on the sync ring, so the
            # FIFO delivers them after all xw bytes (needed only mid-kernel)
            extt = per.tile([128, EXTW], DTB, tag="ext")
            nc.sync.dma_start(out=extt, in_=ext[:, :])
            mA = extt[:, 0:SW2]
            mM = extt[:, SW2:2 * SW2]
            pw_t = [extt[:, 2 * SW2:2 * SW2 + C],
                    extt[:, 2 * SW2 + C:2 * SW2 + 2 * C]]
            # warm-up matmuls on memset data (no DMA dependency): keeps the
            # PE busy while the first x/w chunks stream in, so HAM
            # un-throttles the clock early. wusrc memsets FIRST so the
            # first warm-up LDWEIGHTS is not gated on the other constants.
            wusrc = per.tile([128, 256], F32, tag="wusrc")
            nc.vector.memset(wusrc, 1.0)
            wusrc16 = per.tile([128, 128], DTA, tag="wusrc16")
            nc.vector.memset(wusrc16, 1.0)
            onecol = per.tile([128, 1], F32, tag="onecol")
            nc.vector.memset(onecol, 1.0)
            zpad = per.tile([128, TP - T], F32, tag="zpad")
            nc.vector.memset(zpad, 0.0)
            # NWARM f32 N=256 matmuls (~1.3us each at the cold clock) bridge
            # the PE from the preamble to kc0's arrival (~9.7us on the sync
            # ring). More warm-ups would queue AHEAD of phase 1 on the
            # in-order Tensor engine and delay it past its data.
            wu = pp.tile([128, 256], F32, tag="mm", name="wu", bufs=3)
            for _ in range(NWARM):
                nc.tensor.matmul(out=wu, lhsT=wusrc[:, 0:128], rhs=wusrc,
                                 start=True, stop=True)
            # one short tail warm-up: shrinks the PE gap before kc0 lands
            # (a ~1us gap here sometimes resets the HAM ramp and phase 1
            # then runs at half clock for several microseconds)
            nc.tensor.matmul(out=wu[:, 0:128], lhsT=wusrc[:, 0:128],
                             rhs=wusrc[:, 0:128], start=True, stop=True)

            # ---- phase 1: q,k projection (weight-stationary) ----
            # qkT[mc] rows: mc 0,1 = q (heads 0,1 / 2,3); mc 2,3 = k
            # mc alternates innermost so consecutive matmuls load different
            # weights (LDWEIGHTS overlaps via the background weight buffer).
            qkT = []
            for mc4 in range(4):
                t = per.tile([128, TP], DTA, tag=f"qkT{mc4}", name=f"qkT{mc4}")
                nc.vector.tensor_copy(out=t[:, T:TP], in_=zpad)
                qkT.append(t)
            # ni inner: 8 qkps tiles live across all 8 banks, and each
            # xw chunk is consumed by 8 back-to-back matmuls -- the sweep
            # rate then matches the input-DMA arrival rate (no mid-phase
            # PE stall -> no HAM re-throttle).
            gtag = {(0, 0): "mm", (0, 1): "mm", (1, 0): "mm", (1, 1): "st",
                    (2, 0): "st", (2, 1): "st", (3, 0): "zt", (3, 1): "zt"}
            qps = {}
            for mc in range(4):
                for ni, (n0, nw) in enumerate(QKCH):
                    qps[(mc, ni)] = pp.tile(
                        [128, nw], F32, tag=gtag[(mc, ni)],
                        name=f"qkps{mc}_{ni}", bufs=3 if gtag[(mc, ni)] != "zt" else 2)
            for kc in range(KC):
                for ni, (n0, nw) in enumerate(QKCH):
                    for mc in range(4):
                        nc.tensor.matmul(
                            out=qps[(mc, ni)],
                            lhsT=wqk_t[kc][:, mc * 128:(mc + 1) * 128],
                            rhs=xt[kc][:, n0:n0 + nw],
                            start=(kc == 0), stop=(kc == KC - 1),
                        )
            # pair-0's q,k tiles (mc 0 and 2) evacuate FIRST on both
            # engines: the first score matmuls block the in-order PE queue
            # until qkT[0] and qkT[2] are fully staged, so their evac order
            # sets when the whole attention pipeline starts.
            for mc in (0, 2, 1, 3):
                for ni, (n0, nw) in enumerate(QKCH):
                    if (mc + ni) % 2 == 0:
                        nc.scalar.copy(out=qkT[mc][:, n0:n0 + nw],
                                       in_=qps[(mc, ni)])
                    else:
                        nc.vector.tensor_copy(out=qkT[mc][:, n0:n0 + nw],
                                              in_=qps[(mc, ni)])

            # ---- phase 2: v projection (x-stationary: v in (tokens, dims)) ----
            vone = {}  # (b, jc) -> [th, 4, 65] tile (per-head v cols + ones col)

            def emit_v(b, jc):
                t0, th = TOKCH[jc]
                vps = pp.tile([th, CPC], F32, tag="mm", name=f"vps{b}_{jc}", bufs=3)
                for kc in range(KC):
                    nc.tensor.matmul(
                        out=vps,
                        lhsT=xt[kc][:, b * N + t0: b * N + t0 + th],
                        rhs=wv_t[kc],
                        start=(kc == 0), stop=(kc == KC - 1),
                    )
                vt = per.tile([th, HPC, HD + 1], DTA, tag=f"vone{b}_{jc}",
                              name=f"vone{b}_{jc}")
                nc.vector.tensor_copy(
                    out=vt[:, :, 0:HD],
                    in_=vps.rearrange("t (h d) -> t h d", h=HPC))
                nc.vector.tensor_copy(
                    out=vt[:, :, HD],
                    in_=onecol[:th, 0:1].to_broadcast((th, HPC)))
                vone[(b, jc)] = vt



            # ---- phase 3 + 4: windowed attention, proj interleaved ----
            # per (b,h): ST packed [128, 272]; logits = (ST + A) * M
            # E = exp(logits); zT' = [v|1]^T @ E  (row HD = softmax denom)
            # After batch b completes: project its 197 columns (spread over
            # the next batch's head slots).
            YT = [per.tile([128, T], DTB, tag=f"YT{k2}", name=f"YT{k2}")
                  for k2 in range(2)]

            def attn_s(b, p):
                """score matmuls for head pair (2p, 2p+1) -> two st tiles.
                Each head gets its OWN tile: the two heads contract on
                different PE row groups (partitions 0:64 vs 64:128), and
                different-row-group matmuls draining into the same PSUM
                bank abort the device."""
                st = [pp.tile([128, SW], F32, tag="st",
                              name=f"st{b}_{p}_{s2}", bufs=3)
                      for s2 in range(2)]
                if b >= 2:
                    # keep-warm dummy: HAM re-throttles the PE clock to
                    # 1.2GHz when the late phase goes sparse; this is
                    # overwritten by the real start=True matmuls below
                    nc.tensor.matmul(out=st[0][:, 0:64],
                                     lhsT=wusrc16, rhs=wusrc16[:, 0:64],
                                     start=True, stop=True)
                # blk outer / head inner: adjacent matmuls use different PE
                # row groups AND different PSUM banks, so they execute
                # concurrently in the array (per-subarray concurrency)
                for (j0, jh, i0, iw, c0) in BLOCKS:
                    for s2 in range(2):
                        qTh = qkT[p][64 * s2:64 * s2 + 64, :]
                        kTh = qkT[2 + p][64 * s2:64 * s2 + 64, :]
                        nc.tensor.matmul(
                            out=st[s2][0:jh, c0:c0 + iw],
                            lhsT=kTh[:, b * N + j0: b * N + j0 + jh],
                            rhs=qTh[:, b * N + i0: b * N + i0 + iw],
                            start=True, stop=True,
                        )
                return st

            def attn_rest(b, p, st):
                """mask, exp, AV, normalize for head pair (2p, 2p+1).
                The per-head PSUM score tiles merge into one wide SBUF tile
                at the first (1x-rate PSUM-read) op; everything downstream
                runs once per pair. The AV matmuls are full-array, so the
                packed zt bank is safe."""
                sa = wk.tile([128, SW2], DTA, tag="sa")
                for s2 in range(2):
                    nc.vector.tensor_tensor(
                        sa[:, s2 * SW:(s2 + 1) * SW], st[s2], mA[:, 0:SW],
                        mybir.AluOpType.add)
                sm = wk.tile([128, SW2], DTA, tag="sm")
                nc.vector.tensor_tensor(sm, sa, mM, mybir.AluOpType.mult)
                e = wk.tile([128, SW2], DTA, tag="e")
                nc.scalar.activation(
                    out=e, in_=sm, func=mybir.ActivationFunctionType.Exp)
                last = (b, p) == (3, 1)
                denp = None
                if last:
                    # the final chain's exp->AV wait is the one PE hole with
                    # no real work left to fill; these depend on sm so they
                    # run exactly inside it and keep HAM from re-throttling
                    # the clock for the tail's proj matmuls.
                    wuf = pp.tile([128, 64], F32, tag="mm", name="wuf",
                                  bufs=3)
                    for _ in range(6):
                        nc.tensor.matmul(out=wuf, lhsT=sm[0:128, 0:128],
                                         rhs=sm[0:128, 0:64],
                                         start=True, stop=True)
                    # early denominator (ones^T @ E) as 1-row matmuls queued
                    # AHEAD of the AV matmuls: the dent->reciprocal->
                    # broadcast path then overlaps AV instead of trailing it
                    # on the fully-exposed final serial path.
                    denp = pp.tile([1, SW2], F32, tag="zt", name="den31")
                    for s2 in range(2):
                        for blk, (j0, jh, i0, iw, c0) in enumerate(BLOCKS):
                            nc.tensor.matmul(
                                out=denp[:, s2 * SW + i0: s2 * SW + i0 + iw],
                                lhsT=vone[(b, blk)][0:jh, 2 * p + s2,
                                                    HD:HD + 1],
                                rhs=e[0:jh, s2 * SW + c0: s2 * SW + c0 + iw],
                                start=True, stop=True)
                zt = pp.tile([HD + 1, SW2], F32, tag="zt", name=f"zt{b}_{p}")
                first = True
                for s2 in range(2):
                    for blk, (j0, jh, i0, iw, c0) in enumerate(BLOCKS):
                        nc.tensor.matmul(
                            out=zt[:, s2 * SW + i0: s2 * SW + i0 + iw],
                            lhsT=vone[(b, blk)][0:jh, 2 * p + s2, :],
                            rhs=e[0:jh, s2 * SW + c0: s2 * SW + c0 + iw],
                            start=first, stop=(s2 == 1 and blk == 1),
                        )
                        first = False
                # stage only the denominator row to SBUF (the custom-DVE
                # reciprocal cannot read PSUM on HW); z is normalized
                # straight out of PSUM. Pad columns hold den=128 (mask
                # makes e=1 there) so the reciprocal stays finite.
                dent = wk.tile([1, SW2], F32, tag="dent")
                nc.scalar.copy(out=dent,
                               in_=denp if last else zt[HD:HD + 1, :])
                rrow = wk.tile([1, SW2], F32, tag="rrow")
                nc.vector.reciprocal_approx_fast(out=rrow, in_=dent)
                rb = wk.tile([64, SW2], F32, tag="rb")
                nc.gpsimd.partition_broadcast(rb, rrow)
                if last:
                    # split the final YT write by token range: chunk 5's
                    # k2=1 matmuls (tokens 591:719) unblock after the first
                    # piece of each head instead of the full row
                    for i0, i1 in ((0, 128), (128, N)):
                        for s2 in range(2):
                            nc.vector.tensor_tensor(
                                YT[p][s2 * 64:(s2 + 1) * 64,
                                      b * N + i0:b * N + i1],
                                zt[0:HD, s2 * SW + i0:s2 * SW + i1],
                                rb[:, s2 * SW + i0:s2 * SW + i1],
                                mybir.AluOpType.mult)
                else:
                    for s2 in range(2):
                        nc.vector.tensor_tensor(
                            YT[p][s2 * 64:(s2 + 1) * 64, b * N:(b + 1) * N],
                            zt[0:HD, s2 * SW:s2 * SW + N],
                            rb[:, s2 * SW:s2 * SW + N], mybir.AluOpType.mult)

            obt = {}
            for ci in range(len(PROJC0)):
                obt[ci] = per.tile([CW, C], DTB, tag="ob",
                                   name=f"ob{ci}", bufs=3)

            pj_pend = {}

            def proj_units(units, tag, warm=False, k2s=(0, 1), ptag="mm"):
                # units arrive as (ci, ns) pairs with even ns: each call
                # handles (ns, ns+1) with k2 outer so the YT weight tile is
                # reused by two consecutive matmuls. k2s allows splitting
                # the accumulation across two call sites (tail shortening:
                # the k2=0 half only needs YT[0] = head pair 0).
                for ui, (ci, ns) in enumerate(units):
                    t0 = PROJC0[ci]
                    tsl = slice(t0, t0 + CW)
                    if 0 in k2s:
                        pps = [pp.tile([CW, 512], F32, tag=ptag,
                                       name=f"pj{tag}_{ci}_{ns + u}", bufs=3)
                               for u in range(2)]
                        if 1 not in k2s:
                            pj_pend[(ci, ns)] = pps
                        if warm and ui == 0:
                            # keep-warm dummy: HAM re-throttles when the late
                            # phase goes sparse; overwritten by the start=True
                            # matmuls below
                            nc.tensor.matmul(out=pps[0][0:CW, 0:64],
                                             lhsT=wusrc16, rhs=wusrc16[:, 0:64],
                                             start=True, stop=True)
                    else:
                        pps = pj_pend.pop((ci, ns))
                    for k2 in k2s:
                        for u in range(2):
                            nc.tensor.matmul(
                                out=pps[u],
                                lhsT=YT[k2][:, tsl],
                                rhs=pw_t[k2][:, (ns + u) * 512:(ns + u + 1) * 512],
                                start=(k2 == 0), stop=(k2 == 1),
                            )
                    if 1 not in k2s:
                        continue
                    ob = obt[ci]
                    for u in range(2):
                        osl = slice((ns + u) * 512, (ns + u + 1) * 512)
                        # 3:1 toward ScalarE mid-kernel (VectorE carries the
                        # chain ops); 1:1 for the late chunks where the
                        # evacuations are the critical path to the last DMA
                        # (GpSimd cannot read PSUM)
                        if u == 1 and (ci % 2 == 1 or ci >= 4):
                            nc.vector.tensor_copy(out=ob[:, osl], in_=pps[u])
                        else:
                            nc.scalar.copy(out=ob[:, osl], in_=pps[u])
                    csl = slice(ns * 512, (ns + 2) * 512)
                    nc.sync.dma_start(out=out_d[tsl, csl], in_=ob[:, csl])

            # software-pipeline: emit S matmuls one (b,h) ahead so the PE has
            # independent work while the previous chain's DVE/ACT stages run.
            # Fillers per head slot keep the PE dense: v-projections for the
            # next batches, and ready proj token chunks. Chunk ci is placed
            # in the earliest slot whose completed chains cover its tokens.
            bps = [(b, p) for b in range(B) for p in range(2)]
            vfill = [(1, 0), (1, 1), (2, 0), (2, 1), (3, 0), (3, 1)]
            # chunk ci runs one slot AFTER the last chain covering its
            # tokens: a same-slot chunk's k2=1 matmuls would block the
            # in-order PE queue on the just-emitted chain's YT write (a
            # ~2.5us PE hole that also makes HAM re-throttle the clock).
            psched = {(1, 0): 0, (2, 0): 1, (2, 1): 2}
            # first chain's scores before the v matmuls: its mask/exp stages
            # then overlap the v projection on the PE
            sts = {bps[0]: attn_s(*bps[0])}
            emit_v(0, 0)
            emit_v(0, 1)
            for idx, (b, p) in enumerate(bps):
                if idx + 1 < len(bps):
                    sts[bps[idx + 1]] = attn_s(*bps[idx + 1])
                if (b, p) == (3, 0):
                    # queued on the PE ahead of this chain's AV matmuls:
                    # chunks 3+4 (deps >= 1 slot old) fill the exp wait,
                    # and their Scalar/Vector evacuations clear those
                    # queues before the FINAL chain's mask/exp ops need
                    # them.
                    proj_units([(3, 0), (3, 2)], "c3", warm=True)
                    proj_units([(4, 0), (4, 2)], "c4")
                if (b, p) == (3, 1):
                    # the k2=0 halves of chunk 5 (need only YT[0] <- chain
                    # (3,0)) run while the final chain's mask/exp stages
                    # execute. The pending PSUM tiles split across the "mm"
                    # pool (2) and the "st" pool (2: one free buf now, one
                    # released by the chain's first mask-add).
                    proj_units([(5, 0)], "c5a", k2s=(0,))
                    proj_units([(5, 2)], "c5b", k2s=(0,), ptag="st")
                attn_rest(b, p, sts.pop((b, p)))
                if vfill:
                    emit_v(*vfill.pop(0))
                if (b, p) in psched:
                    ci = psched[(b, p)]
                    proj_units([(ci, ns) for ns in range(0, NS, 2)],
                               f"c{ci}", warm=(b >= 2))
            proj_units([(5, 0)], "c5a", k2s=(1,))
            proj_units([(5, 2)], "c5b", k2s=(1,))
            proj_units([(6, ns) for ns in range(0, NS, 2)], "c6")
            # hold the PE-activity signal into the final DMA drain: HAM
            # re-throttles the clock ~3.4us after the PE idles, halving the
            # teardown sequencer rate. These read the last output tile so
            # they execute during the drain; they end before the drain does
            # (the useful-time window is set by the last DMA, not by them).
            wud = pp.tile([128, 64], F32, tag="mm", name="wud", bufs=3)
            for _ in range(12):
                nc.tensor.matmul(out=wud, lhsT=obt[6][:, 1536:1664],
                                 rhs=obt[6][:, 0:64], start=True, stop=True)

    nc.compile()
    return nc


def _host_masks(np_att):
    i = np.arange(N)[:, None]
    j = np.arange(N)[None, :]
    d = np.abs(i - j).astype(np.float32)
    in_win = (j >= i - WIN) & (j < i + WIN)
    m = np.where(in_win, (WIN - d / 2.0) / WIN, 0.0).astype(np.float32)
    # transposed (j on rows): logits[j,i] = (ST[j,i] + A[j,i]) * M[j,i]
    multT = np.where(in_win, m * SCALE, 1.0).astype(np.float32).T
    addT = np.where(in_win, 0.0, NEG).astype(np.float32).T
    # pack the two banded blocks side by side into [128, SW] tiles
    mult = np.zeros((128, SW), dtype=np.float32)
    addm = np.zeros((128, SW), dtype=np.float32)
    for blk, (j0, jh, i0, iw, c0) in enumerate(BLOCKS):
        iw_r = min(iw, N - i0)  # data columns (rest stays pad)
        mult[0:jh, c0:c0 + iw_r] = multT[j0:j0 + jh, i0:i0 + iw_r]
        addm[0:jh, c0:c0 + iw_r] = addT[j0:j0 + jh, i0:i0 + iw_r]
    # duplicate for the two heads packed side by side; pad cells stay
    # (M=0, A=0) so e = exp(0) = 1 and denominators remain finite.
    return (np.tile(addm, (1, 2)).astype(np_att),
            np.tile(mult, (1, 2)).astype(np_att))


def _np_dt(name):
    if name == "bf16":
        import ml_dtypes
        return ml_dtypes.bfloat16
    if name == "fp16":
        return np.float16
    return np.float32


def _make_in_maps(x, qkv_w, proj_w):
    npb = _np_dt(DT_BIG)
    npa = _np_dt(DT_ATT)
    xT = x.reshape(T, C).T
    addm, mult = _host_masks(npa)
    in_maps = []
    for d in range(NCORES):
        r = slice(d * CPC, (d + 1) * CPC)
        wqk_d = np.concatenate(
            [qkv_w[r, :], qkv_w[C + d * CPC: C + (d + 1) * CPC, :]], axis=0).T
        wv_d = qkv_w[2 * C + d * CPC: 2 * C + (d + 1) * CPC, :].T
        xw_d = np.ascontiguousarray(
            np.concatenate([xT, wqk_d, wv_d], axis=1)).astype(npb)
        pw_d = proj_w[:, r].T.astype(npb)
        ext_d = np.ascontiguousarray(np.concatenate(
            [addm.astype(npb), mult.astype(npb),
             pw_d[0:128, :], pw_d[128:256, :]], axis=1))
        in_maps.append({"xw": xw_d, "ext": ext_d})
    return in_maps


def kernel(x, qkv_w, proj_w, proj_b):
    from concourse.bass_utils import run_bass_kernel_spmd

    key = (DT_BIG, DT_ATT)
    if key not in _compiled:
        _compiled[key] = _build_program(*key)
    nc = _compiled[key]

    x = np.asarray(x, dtype=np.float32)
    qkv_w = np.asarray(qkv_w, dtype=np.float32)
    proj_w = np.asarray(proj_w, dtype=np.float32)
    proj_b = np.asarray(proj_b, dtype=np.float32)

    in_maps = _make_in_maps(x, qkv_w, proj_w)
    res = run_bass_kernel_spmd(nc, in_maps, core_ids=list(range(NCORES)))
    acc = np.zeros((T, C), dtype=np.float32)
    for r in res.results:
        acc += r["out"].astype(np.float32)
    out = acc + proj_b[None, :]
    return np.ascontiguousarray(out).reshape(B, N, C)

